# revision 14
# baseline (speedup 1.0000x reference)
"""Trainium2 Bass kernel for nn_CellularGNN (2-layer GCN + mean-pool + linear head).

Strategy (8 NeuronCores, SPMD, dst-partitioned):
  - Nodes partitioned contiguously across cores (12500 each, padded to 12544);
    each core owns the edges whose DESTINATION lands in its range.
  - GCN algebra refactored so both edge aggregations run in 32-channel space:
    u = dinv * h;  out[d] = dinv[d]*(sum_{e->d} u[src_e]) + b  with the self
    loop appended as an explicit edge, and layer 2's weight matmul applied
    after aggregation (aggregation commutes with right-multiplication).
  - Per layer, the 32-wide u table is AllGathered so every core can gather
    any source row; rows are fetched 128 at a time with the (proven)
    one-index-per-partition indirect DMA.
  - Aggregation per 128-dst-node window: one-hot matrices built on the vector
    engine from local dst offsets; PE matmul (lhsT = gathered rows [128e,32],
    rhs = one-hot [128e,128n]) accumulated in PSUM -> S^T [32,128].
  - Mean-pool partials per graph via a one-hot matmul accumulated over
    windows, then a tiny AllReduce and the classifier head.

kernel(**inputs) takes the FULL unsharded inputs and returns (logits, pooled).
"""

import sys
from contextlib import ExitStack

import numpy as np

for _p in ("/opt/trn_rl_repo", "/opt/pypackages"):
    if _p not in sys.path:
        sys.path.append(_p)

import concourse.bacc as bacc
import concourse.bass as bass
import concourse.tile as tile
from concourse import mybir
from concourse.bass import IndirectOffsetOnAxis
from concourse.bass_utils import run_bass_kernel_spmd

F32 = mybir.dt.float32
I32 = mybir.dt.int32
AF = mybir.ActivationFunctionType
ALU = mybir.AluOpType


def full_cfg():
    return dict(N=100000, E=1600000, G=64, IN=128, C1=32, C2=64, CORES=8)


def derived(cfg):
    P = dict(cfg)
    assert P["N"] % P["CORES"] == 0
    P["NPC"] = P["N"] // P["CORES"]          # real nodes per core
    P["TW"] = (P["NPC"] + 127) // 128        # node windows per core
    P["NPCP"] = P["TW"] * 128                # padded nodes per core
    P["NTOT"] = P["CORES"] * P["NPCP"]       # padded global node count
    return P


# ----------------------------------------------------------------------------
# Host-side preprocessing
# ----------------------------------------------------------------------------

def preprocess(inputs, P):
    x = np.asarray(inputs["x"], dtype=np.float32)
    W1 = np.asarray(inputs["W1"], dtype=np.float32)
    b1 = np.asarray(inputs["b1"], dtype=np.float32)
    W2 = np.asarray(inputs["W2"], dtype=np.float32)
    b2 = np.asarray(inputs["b2"], dtype=np.float32)
    Wc = np.asarray(inputs["Wc"], dtype=np.float32)
    bc = np.asarray(inputs["bc"], dtype=np.float32)
    edge_index = np.asarray(inputs["edge_index"])
    batch = np.asarray(inputs["batch"])

    N, CORES, NPC, TW, NPCP = P["N"], P["CORES"], P["NPC"], P["TW"], P["NPCP"]
    G, IN, C1, C2 = P["G"], P["IN"], P["C1"], P["C2"]

    src = edge_index[0].astype(np.int64)
    dst = edge_index[1].astype(np.int64)
    deg = (np.bincount(dst, minlength=N) + 1).astype(np.float32)

    srcg_all = ((src // NPC) * NPCP + (src % NPC)).astype(np.int32)
    core_of = (dst // NPC).astype(np.int64)
    local = dst - core_of * NPC
    w_of = local // 128
    dl_of = local % 128

    per_core = []
    counts = np.zeros((CORES, TW), dtype=np.int64)
    for c in range(CORES):
        m = core_of == c
        ln = np.arange(NPC, dtype=np.int64)  # self edges for owned nodes
        allsrc = np.concatenate([srcg_all[m],
                                 (c * NPCP + ln).astype(np.int32)])
        allw = np.concatenate([w_of[m], ln // 128])
        alldl = np.concatenate([dl_of[m], ln % 128])
        order = np.argsort(allw, kind="stable")
        sw = allw[order]
        counts[c] = np.bincount(sw, minlength=TW)
        per_core.append((allsrc[order], sw, alldl[order]))

    # shared static tiles-per-window
    ET_w = np.maximum(1, (counts.max(axis=0) + 127) // 128).astype(np.int64)
    tile_base = np.zeros(TW + 1, dtype=np.int64)
    np.cumsum(ET_w, out=tile_base[1:])
    TT = int(tile_base[-1])
    S = TT * 128
    P = dict(P)
    P["ET_key"] = hash(ET_w.tobytes())
    P["ET_w"] = ET_w
    P["tile_base"] = tile_base
    P["TT"] = TT

    iotaC = np.broadcast_to(np.arange(128, dtype=np.float32), (128, 128)).copy()
    iotaG = np.broadcast_to(np.arange(G, dtype=np.float32), (128, G)).copy()
    ident = np.eye(128, dtype=np.float32)
    b1r = np.broadcast_to(b1, (128, C1)).copy()
    b2r = np.broadcast_to(b2, (128, C2)).copy()
    bcr = np.broadcast_to(bc, (G, 2)).copy()

    slot_start = tile_base * 128

    in_maps = []
    for c in range(CORES):
        allsrc, sw, alldl = per_core[c]
        starts = np.zeros(TW, dtype=np.int64)
        cc = counts[c]
        np.cumsum(cc[:-1], out=starts[1:])
        posin = np.arange(len(sw)) - starts[sw]
        flat_src = np.zeros(S, dtype=np.int32)
        flat_dl = np.full(S, 255.0, dtype=np.float32)
        slots = slot_start[sw] + posin
        flat_src[slots] = allsrc
        flat_dl[slots] = alldl
        # [p, tile] layout: slot (tile, p) = flat[tile*128 + p]
        srcidx = np.ascontiguousarray(flat_src.reshape(TT, 128).T)
        dstl = np.ascontiguousarray(flat_dl.reshape(TT, 128).T)

        xc = x[c * NPC:(c + 1) * NPC]
        xT = np.zeros((IN, NPCP), dtype=np.float32)
        xT[:, :NPC] = xc.T

        degc = np.ones(NPCP, dtype=np.float32)
        degc[:NPC] = deg[c * NPC:(c + 1) * NPC]
        degt = np.ascontiguousarray(degc.reshape(TW, 128).T)

        bt = np.full(NPCP, -1.0, dtype=np.float32)
        bt[:NPC] = batch[c * NPC:(c + 1) * NPC].astype(np.float32)
        batcht = np.ascontiguousarray(bt.reshape(TW, 128).T)

        in_maps.append(dict(
            xT=xT, srcidx=srcidx, dstl=dstl, deg=degt, batchid=batcht,
            W1=W1, W2=W2, Wc=Wc, b1r=b1r, b2r=b2r, bcr=bcr,
            iotaC=iotaC, iotaG=iotaG, ident=ident,
        ))
    return in_maps, P


# ----------------------------------------------------------------------------
# Device program
# ----------------------------------------------------------------------------

def build_program(P, enable_asserts=False):
    CORES, TW, NPCP, NTOT = P["CORES"], P["TW"], P["NPCP"], P["NTOT"]
    IN, C1, C2, G = P["IN"], P["C1"], P["C2"], P["G"]
    ET_w, tile_base, TT = P["ET_w"], P["tile_base"], P["TT"]
    ETMAX = int(max(ET_w))

    nc = bacc.Bacc("TRN2", target_bir_lowering=False, debug=False,
                   enable_asserts=enable_asserts, num_devices=CORES)

    d_xT = nc.dram_tensor("xT", [IN, NPCP], F32, kind="ExternalInput").ap()
    d_srcidx = nc.dram_tensor("srcidx", [128, TT], I32,
                              kind="ExternalInput").ap()
    d_dstl = nc.dram_tensor("dstl", [128, TT], F32, kind="ExternalInput").ap()
    d_deg = nc.dram_tensor("deg", [128, TW], F32, kind="ExternalInput").ap()
    d_batch = nc.dram_tensor("batchid", [128, TW], F32,
                             kind="ExternalInput").ap()
    d_W1 = nc.dram_tensor("W1", [IN, C1], F32, kind="ExternalInput").ap()
    d_W2 = nc.dram_tensor("W2", [C1, C2], F32, kind="ExternalInput").ap()
    d_Wc = nc.dram_tensor("Wc", [C2, 2], F32, kind="ExternalInput").ap()
    d_b1r = nc.dram_tensor("b1r", [128, C1], F32, kind="ExternalInput").ap()
    d_b2r = nc.dram_tensor("b2r", [128, C2], F32, kind="ExternalInput").ap()
    d_bcr = nc.dram_tensor("bcr", [G, 2], F32, kind="ExternalInput").ap()
    d_iotaC = nc.dram_tensor("iotaC", [128, 128], F32,
                             kind="ExternalInput").ap()
    d_iotaG = nc.dram_tensor("iotaG", [128, G], F32, kind="ExternalInput").ap()
    d_ident = nc.dram_tensor("ident", [128, 128], F32,
                             kind="ExternalInput").ap()
    d_logits = nc.dram_tensor("logits", [G, 2], F32, kind="ExternalOutput").ap()
    d_pooled = nc.dram_tensor("pooled", [G, C2], F32,
                              kind="ExternalOutput").ap()

    rg = [list(range(CORES))]

    with tile.TileContext(nc) as tc:
        with ExitStack() as ctx:
            const = ctx.enter_context(tc.tile_pool(name="const", bufs=1))
            dram = ctx.enter_context(tc.tile_pool(name="dram", bufs=1,
                                                  space="DRAM"))

            def load_const(name, ap, shape, dtype=F32):
                t = const.tile(shape, dtype, name=name)
                nc.sync.dma_start(out=t[:, :], in_=ap[:, :])
                return t

            srcidx_sb = load_const("srcidx_sb", d_srcidx, [128, TT], I32)
            dstl_sb = load_const("dstl_sb", d_dstl, [128, TT])
            deg_sb = load_const("deg_sb", d_deg, [128, TW])
            batch_sb = load_const("batch_sb", d_batch, [128, TW])
            W1_sb = load_const("W1_sb", d_W1, [IN, C1])
            W2_sb = load_const("W2_sb", d_W2, [C1, C2])
            Wc_sb = load_const("Wc_sb", d_Wc, [C2, 2])
            b1r_sb = load_const("b1r_sb", d_b1r, [128, C1])
            b2r_sb = load_const("b2r_sb", d_b2r, [128, C2])
            bcr_sb = load_const("bcr_sb", d_bcr, [G, 2])
            iotaC_sb = load_const("iotaC_sb", d_iotaC, [128, 128])
            iotaG_sb = load_const("iotaG_sb", d_iotaG, [128, G])
            ident_sb = load_const("ident_sb", d_ident, [128, 128])

            sq_sb = const.tile([128, TW], F32, name="sq_sb")
            nc.scalar.sqrt(sq_sb[:, :], deg_sb[:, :])
            dinv_sb = const.tile([128, TW], F32, name="dinv_sb")
            nc.vector.reciprocal(dinv_sb[:, :], sq_sb[:, :])

            u1_cc = dram.tile([NPCP, C1], F32, name="u1_cc")
            u1_tab = dram.tile([NTOT, C1], F32, name="u1_tab",
                               addr_space="Shared")
            u2_cc = dram.tile([NPCP, C1], F32, name="u2_cc")
            u2_tab = dram.tile([NTOT, C1], F32, name="u2_tab",
                               addr_space="Shared")
            ar_in = dram.tile([G, C2 + 1], F32, name="ar_in")
            ar_out = dram.tile([G, C2 + 1], F32, name="ar_out")

            # ---------------- Phase A: u1 = dinv * (x @ W1)
            with tc.tile_pool(name="xp", bufs=3) as xp, \
                 tc.tile_pool(name="psA", bufs=4, space="PSUM") as psA, \
                 tc.tile_pool(name="sbA", bufs=4) as sbA:
                for t in range(TW):
                    xt = xp.tile([IN, 128], F32, name="xt", tag="xt")
                    nc.sync.dma_start(out=xt[:, :],
                                      in_=d_xT[:, t * 128:(t + 1) * 128])
                    hp = psA.tile([128, C1], F32, name="hp", tag="hp")
                    nc.tensor.matmul(out=hp[:, :], lhsT=xt[:, :],
                                     rhs=W1_sb[:, :], start=True, stop=True)
                    u1t = sbA.tile([128, C1], F32, name="u1t", tag="u1t")
                    nc.vector.tensor_scalar_mul(u1t[:, :], hp[:, :],
                                                dinv_sb[:, t:t + 1])
                    nc.sync.dma_start(out=u1_cc[t * 128:(t + 1) * 128, :],
                                      in_=u1t[:, :])

            nc.gpsimd.collective_compute(
                "AllGather", ALU.bypass, replica_groups=rg,
                ins=[u1_cc.opt()], outs=[u1_tab.opt()])

            # ---------------- shared per-layer edge aggregation
            def layer_pass(tab, flush_cb, tag):
                with tc.tile_pool(name=f"gp{tag}", bufs=8) as gpool, \
                     tc.tile_pool(name=f"mp{tag}", bufs=3) as mpool, \
                     tc.tile_pool(name=f"pst{tag}", bufs=2, space="PSUM") as pst, \
                     tc.tile_pool(name=f"fl{tag}_ps1", bufs=2, space="PSUM") as fps1, \
                     tc.tile_pool(name=f"fl{tag}_ps2", bufs=2, space="PSUM") as fps2, \
                     tc.tile_pool(name=f"fl{tag}_sb", bufs=4) as fsb:
                    for w in range(TW):
                        et = int(ET_w[w])
                        wtb = int(tile_base[w])
                        Msb = mpool.tile([128, ETMAX * 128], F32,
                                         name="Msb", tag="M")
                        gts = []
                        for k in range(et):
                            gt = gpool.tile([128, C1], F32, name="gt", tag="g")
                            nc.gpsimd.indirect_dma_start(
                                out=gt[:, :], out_offset=None, in_=tab,
                                in_offset=IndirectOffsetOnAxis(
                                    ap=srcidx_sb[:, wtb + k:wtb + k + 1],
                                    axis=0))
                            gts.append(gt)
                            nc.vector.tensor_scalar(
                                out=Msb[:, k * 128:(k + 1) * 128],
                                in0=iotaC_sb[:, :],
                                scalar1=dstl_sb[:, wtb + k:wtb + k + 1],
                                scalar2=None, op0=ALU.is_equal)
                        ST = pst.tile([C1, 128], F32, name="ST", tag="ST")
                        for k in range(et):
                            nc.tensor.matmul(
                                out=ST[:, :], lhsT=gts[k][:, :],
                                rhs=Msb[:, k * 128:(k + 1) * 128],
                                start=(k == 0), stop=(k == et - 1))
                        flush_cb(w, ST, fps1, fps2, fsb)

            # ---------------- layer 1 flush -> u2
            def flush1(w, ST, fps1, fps2, fsb):
                stsb = fsb.tile([C1, 128], F32, name="stsb", tag="stsb")
                nc.scalar.copy(stsb[:, :], ST[:, :])
                Sp = fps1.tile([128, C1], F32, name="Sp", tag="Sp")
                nc.tensor.transpose(Sp[:, :], stsb[:, :], ident_sb[:C1, :C1])
                h1p = fsb.tile([128, C1], F32, name="h1p", tag="h1p")
                nc.vector.scalar_tensor_tensor(
                    out=h1p[:, :], in0=Sp[:, :], scalar=dinv_sb[:, w:w + 1],
                    in1=b1r_sb[:, :], op0=ALU.mult, op1=ALU.add)
                u2t = fsb.tile([128, C1], F32, name="u2t", tag="u2t")
                nc.scalar.activation(u2t[:, :], h1p[:, :], AF.Relu,
                                     scale=dinv_sb[:, w:w + 1])
                nc.sync.dma_start(out=u2_cc[w * 128:(w + 1) * 128, :],
                                  in_=u2t[:, :])

            layer_pass(u1_tab, flush1, "1")

            nc.gpsimd.collective_compute(
                "AllGather", ALU.bypass, replica_groups=rg,
                ins=[u2_cc.opt()], outs=[u2_tab.opt()])

            # ---------------- layer 2 flush -> h2 -> pooling
            with tc.tile_pool(name="sbacc", bufs=1) as sbacc, \
                 tc.tile_pool(name="pspw", bufs=2, space="PSUM") as pspw:
                pacc_sb = sbacc.tile([G, C2 + 1], F32, name="pacc_sb")
                nc.vector.memset(pacc_sb[:, :], 0.0)

                def flush2(w, ST, fps1, fps2, fsb):
                    stsb2 = fsb.tile([C1, 128], F32, name="stsb2", tag="stsb2")
                    nc.scalar.copy(stsb2[:, :], ST[:, :])
                    h2p = fps1.tile([128, C2], F32, name="h2p", tag="h2p")
                    nc.tensor.matmul(out=h2p[:, :], lhsT=stsb2[:, :],
                                     rhs=W2_sb[:, :], start=True, stop=True)
                    h2pre = fsb.tile([128, C2], F32, name="h2pre", tag="h2pre")
                    nc.vector.scalar_tensor_tensor(
                        out=h2pre[:, :], in0=h2p[:, :],
                        scalar=dinv_sb[:, w:w + 1], in1=b2r_sb[:, :],
                        op0=ALU.mult, op1=ALU.add)
                    h2e = fsb.tile([128, C2 + 1], F32, name="h2e", tag="h2e")
                    nc.scalar.activation(h2e[:, :C2], h2pre[:, :], AF.Relu)
                    nc.vector.memset(h2e[:, C2:C2 + 1], 1.0)
                    Bsb = fsb.tile([128, G], F32, name="Bsb", tag="Bsb")
                    nc.vector.tensor_scalar(
                        out=Bsb[:, :], in0=iotaG_sb[:, :],
                        scalar1=batch_sb[:, w:w + 1], scalar2=None,
                        op0=ALU.is_equal)
                    pw = pspw.tile([G, C2 + 1], F32, name="pw", tag="pw")
                    nc.tensor.matmul(out=pw[:, :], lhsT=Bsb[:, :],
                                     rhs=h2e[:, :], start=True, stop=True)
                    nc.vector.tensor_add(pacc_sb[:, :], pacc_sb[:, :],
                                         pw[:, :])

                layer_pass(u2_tab, flush2, "2")

                nc.sync.dma_start(out=ar_in[:, :], in_=pacc_sb[:, :])

            nc.gpsimd.collective_compute(
                "AllReduce", ALU.add, replica_groups=rg,
                ins=[ar_in.opt()], outs=[ar_out.opt()])

            # ---------------- Phase F: pooled mean + logits
            with tc.tile_pool(name="sbF", bufs=1) as sbF, \
                 tc.tile_pool(name="psF", bufs=1, space="PSUM") as psF:
                asb = sbF.tile([G, C2 + 1], F32, name="asb")
                nc.sync.dma_start(out=asb[:, :], in_=ar_out[:, :])
                cnt = sbF.tile([G, 1], F32, name="cnt")
                nc.vector.tensor_scalar(out=cnt[:, :], in0=asb[:, C2:C2 + 1],
                                        scalar1=1.0, scalar2=None, op0=ALU.max)
                rec = sbF.tile([G, 1], F32, name="rec")
                nc.vector.reciprocal(rec[:, :], cnt[:, :])
                pooled_sb = sbF.tile([G, C2], F32, name="pooled_sb")
                nc.vector.tensor_scalar(out=pooled_sb[:, :], in0=asb[:, :C2],
                                        scalar1=rec[:, :1], scalar2=None,
                                        op0=ALU.mult)
                nc.sync.dma_start(out=d_pooled[:, :], in_=pooled_sb[:, :])

                pT = psF.tile([C2, G], F32, name="pT")
                nc.tensor.transpose(pT[:, :], pooled_sb[:, :],
                                    ident_sb[:G, :G])
                pT_sb = sbF.tile([C2, G], F32, name="pT_sb")
                nc.scalar.copy(pT_sb[:, :], pT[:, :])
                lg = psF.tile([G, 2], F32, name="lg")
                nc.tensor.matmul(out=lg[:, :], lhsT=pT_sb[:, :],
                                 rhs=Wc_sb[:, :], start=True, stop=True)
                lg_sb = sbF.tile([G, 2], F32, name="lg_sb")
                nc.vector.tensor_add(lg_sb[:, :], lg[:, :], bcr_sb[:, :])
                nc.sync.dma_start(out=d_logits[:, :], in_=lg_sb[:, :])

    nc.compile()
    return nc


# ----------------------------------------------------------------------------
# Entry point
# ----------------------------------------------------------------------------

_PROGRAM_CACHE = {}


def _get_program(P):
    key = (P["N"], P["E"], P["G"], P["IN"], P["C1"], P["C2"], P["CORES"],
           P["ET_key"])
    if key not in _PROGRAM_CACHE:
        _PROGRAM_CACHE[key] = build_program(P)
    return _PROGRAM_CACHE[key]


def run(inputs, cfg=None, trace=False):
    P = derived(cfg or full_cfg())
    in_maps, P = preprocess(inputs, P)
    nc = _get_program(P)
    res = run_bass_kernel_spmd(nc, in_maps, core_ids=list(range(P["CORES"])),
                               trace=trace)
    logits = np.asarray(res.results[0]["logits"], dtype=np.float32)
    pooled = np.asarray(res.results[0]["pooled"], dtype=np.float32)
    return (logits, pooled), res


def kernel(**inputs):
    (logits, pooled), _ = run(inputs)
    return logits, pooled


# revision 20
# speedup vs baseline: 1.0627x; 1.0627x over previous
"""Trainium2 Bass kernel for nn_CellularGNN (2-layer GCN + mean-pool + linear head).

Strategy (8 NeuronCores, SPMD, dst-partitioned):
  - Nodes partitioned contiguously across cores (12500 each, padded to 12544);
    each core owns the edges whose DESTINATION lands in its range.
  - GCN algebra refactored so both edge aggregations run in 32-channel space:
    u = dinv * h;  out[d] = dinv[d]*(sum_{e->d} u[src_e]) + b  with the self
    loop appended as an explicit edge, and layer 2's weight matmul applied
    after aggregation (aggregation commutes with right-multiplication).
  - Per layer, the 32-wide u table is AllGathered so every core can gather
    any source row; rows are fetched 128 at a time with the (proven)
    one-index-per-partition indirect DMA.
  - Aggregation per 128-dst-node window: one-hot matrices built on the vector
    engine from local dst offsets; PE matmul (lhsT = gathered rows [128e,32],
    rhs = one-hot [128e,128n]) accumulated in PSUM -> S^T [32,128].
  - Mean-pool partials per graph via a one-hot matmul accumulated over
    windows, then a tiny AllReduce and the classifier head.

kernel(**inputs) takes the FULL unsharded inputs and returns (logits, pooled).
"""

import sys
from contextlib import ExitStack

import numpy as np

for _p in ("/opt/trn_rl_repo", "/opt/pypackages"):
    if _p not in sys.path:
        sys.path.append(_p)

import concourse.bacc as bacc
import concourse.bass as bass
import concourse.tile as tile
from concourse import mybir
from concourse.bass import IndirectOffsetOnAxis
from concourse.bass_utils import run_bass_kernel_spmd

F32 = mybir.dt.float32
BF16 = mybir.dt.bfloat16
I32 = mybir.dt.int32
AF = mybir.ActivationFunctionType
ALU = mybir.AluOpType


def full_cfg():
    return dict(N=100000, E=1600000, G=64, IN=128, C1=32, C2=64, CORES=8)


def derived(cfg):
    P = dict(cfg)
    assert P["N"] % P["CORES"] == 0
    P["NPC"] = P["N"] // P["CORES"]          # real nodes per core
    P["TW"] = (P["NPC"] + 127) // 128        # node windows per core
    P["NPCP"] = P["TW"] * 128                # padded nodes per core
    P["NTOT"] = P["CORES"] * P["NPCP"]       # padded global node count
    return P


# ----------------------------------------------------------------------------
# Host-side preprocessing
# ----------------------------------------------------------------------------

def preprocess(inputs, P):
    x = np.asarray(inputs["x"], dtype=np.float32)
    W1 = np.asarray(inputs["W1"], dtype=np.float32)
    b1 = np.asarray(inputs["b1"], dtype=np.float32)
    W2 = np.asarray(inputs["W2"], dtype=np.float32)
    b2 = np.asarray(inputs["b2"], dtype=np.float32)
    Wc = np.asarray(inputs["Wc"], dtype=np.float32)
    bc = np.asarray(inputs["bc"], dtype=np.float32)
    edge_index = np.asarray(inputs["edge_index"])
    batch = np.asarray(inputs["batch"])

    N, CORES, NPC, TW, NPCP = P["N"], P["CORES"], P["NPC"], P["TW"], P["NPCP"]
    G, IN, C1, C2 = P["G"], P["IN"], P["C1"], P["C2"]

    src = edge_index[0].astype(np.int64)
    dst = edge_index[1].astype(np.int64)
    deg = (np.bincount(dst, minlength=N) + 1).astype(np.float32)

    srcg_all = ((src // NPC) * NPCP + (src % NPC)).astype(np.int32)
    core_of = (dst // NPC).astype(np.int64)
    local = dst - core_of * NPC
    w_of = local // 128
    dl_of = local % 128

    per_core = []
    counts = np.zeros((CORES, TW), dtype=np.int64)
    for c in range(CORES):
        m = core_of == c
        allsrc = srcg_all[m]
        allw = w_of[m]
        alldl = dl_of[m]
        order = np.argsort(allw, kind="stable")
        sw = allw[order]
        counts[c] = np.bincount(sw, minlength=TW)
        per_core.append((allsrc[order], sw, alldl[order]))

    # shared static tiles-per-window
    ET_w = np.maximum(1, (counts.max(axis=0) + 127) // 128).astype(np.int64)
    tile_base = np.zeros(TW + 1, dtype=np.int64)
    np.cumsum(ET_w, out=tile_base[1:])
    TT = int(tile_base[-1])
    S = TT * 128
    P = dict(P)
    P["ET_key"] = hash(ET_w.tobytes())
    P["ET_w"] = ET_w
    P["tile_base"] = tile_base
    P["TT"] = TT

    iotaC = np.broadcast_to(np.arange(128, dtype=np.float32), (128, 128)).copy()
    iotaG = np.broadcast_to(np.arange(G, dtype=np.float32), (128, G)).copy()
    ident = np.eye(128, dtype=np.float32)
    b1r = np.broadcast_to(b1, (128, C1)).copy()
    b2r = np.broadcast_to(b2, (128, C2)).copy()
    bcr = np.broadcast_to(bc, (G, 2)).copy()

    slot_start = tile_base * 128

    in_maps = []
    for c in range(CORES):
        allsrc, sw, alldl = per_core[c]
        starts = np.zeros(TW, dtype=np.int64)
        cc = counts[c]
        np.cumsum(cc[:-1], out=starts[1:])
        posin = np.arange(len(sw)) - starts[sw]
        flat_src = np.zeros(S, dtype=np.int32)
        flat_dl = np.full(S, 255.0, dtype=np.float32)
        slots = slot_start[sw] + posin
        flat_src[slots] = allsrc
        flat_dl[slots] = alldl
        # [p, tile] layout: slot (tile, p) = flat[tile*128 + p]
        srcidx = np.ascontiguousarray(flat_src.reshape(TT, 128).T)
        dstl = np.ascontiguousarray(flat_dl.reshape(TT, 128).T)

        xc = x[c * NPC:(c + 1) * NPC]
        xT = np.zeros((IN, NPCP), dtype=np.float32)
        xT[:, :NPC] = xc.T

        degc = np.ones(NPCP, dtype=np.float32)
        degc[:NPC] = deg[c * NPC:(c + 1) * NPC]
        degt = np.ascontiguousarray(degc.reshape(TW, 128).T)

        bt = np.full(NPCP, -1.0, dtype=np.float32)
        bt[:NPC] = batch[c * NPC:(c + 1) * NPC].astype(np.float32)
        batcht = np.ascontiguousarray(bt.reshape(TW, 128).T)

        in_maps.append(dict(
            xT=xT, srcidx=srcidx, dstl=dstl, deg=degt, batchid=batcht,
            W1=W1, W2=W2, Wc=Wc, b1r=b1r, b2r=b2r, bcr=bcr,
            iotaC=iotaC, iotaG=iotaG, ident=ident,
        ))
    return in_maps, P


# ----------------------------------------------------------------------------
# Device program
# ----------------------------------------------------------------------------

def build_program(P, enable_asserts=False):
    CORES, TW, NPCP, NTOT = P["CORES"], P["TW"], P["NPCP"], P["NTOT"]
    IN, C1, C2, G = P["IN"], P["C1"], P["C2"], P["G"]
    ET_w, tile_base, TT = P["ET_w"], P["tile_base"], P["TT"]
    ETMAX = int(max(ET_w))

    nc = bacc.Bacc("TRN2", target_bir_lowering=False, debug=False,
                   enable_asserts=enable_asserts, num_devices=CORES)

    d_xT = nc.dram_tensor("xT", [IN, NPCP], F32, kind="ExternalInput").ap()
    d_srcidx = nc.dram_tensor("srcidx", [128, TT], I32,
                              kind="ExternalInput").ap()
    d_dstl = nc.dram_tensor("dstl", [128, TT], F32, kind="ExternalInput").ap()
    d_deg = nc.dram_tensor("deg", [128, TW], F32, kind="ExternalInput").ap()
    d_batch = nc.dram_tensor("batchid", [128, TW], F32,
                             kind="ExternalInput").ap()
    d_W1 = nc.dram_tensor("W1", [IN, C1], F32, kind="ExternalInput").ap()
    d_W2 = nc.dram_tensor("W2", [C1, C2], F32, kind="ExternalInput").ap()
    d_Wc = nc.dram_tensor("Wc", [C2, 2], F32, kind="ExternalInput").ap()
    d_b1r = nc.dram_tensor("b1r", [128, C1], F32, kind="ExternalInput").ap()
    d_b2r = nc.dram_tensor("b2r", [128, C2], F32, kind="ExternalInput").ap()
    d_bcr = nc.dram_tensor("bcr", [G, 2], F32, kind="ExternalInput").ap()
    d_iotaC = nc.dram_tensor("iotaC", [128, 128], F32,
                             kind="ExternalInput").ap()
    d_iotaG = nc.dram_tensor("iotaG", [128, G], F32, kind="ExternalInput").ap()
    d_ident = nc.dram_tensor("ident", [128, 128], F32,
                             kind="ExternalInput").ap()
    d_logits = nc.dram_tensor("logits", [G, 2], F32, kind="ExternalOutput").ap()
    d_pooled = nc.dram_tensor("pooled", [G, C2], F32,
                              kind="ExternalOutput").ap()

    rg = [list(range(CORES))]

    with tile.TileContext(nc) as tc:
        with ExitStack() as ctx:
            const = ctx.enter_context(tc.tile_pool(name="const", bufs=1))
            dram = ctx.enter_context(tc.tile_pool(name="dram", bufs=1,
                                                  space="DRAM"))

            def load_const(name, ap, shape, dtype=F32):
                t = const.tile(shape, dtype, name=name)
                nc.sync.dma_start(out=t[:, :], in_=ap[:, :])
                return t

            srcidx_sb = load_const("srcidx_sb", d_srcidx, [128, TT], I32)
            dstl_sb = load_const("dstl_sb", d_dstl, [128, TT])
            deg_sb = load_const("deg_sb", d_deg, [128, TW])
            batch_sb = load_const("batch_sb", d_batch, [128, TW])
            W1_sb = load_const("W1_sb", d_W1, [IN, C1])
            W2_sb = load_const("W2_sb", d_W2, [C1, C2])
            Wc_sb = load_const("Wc_sb", d_Wc, [C2, 2])
            b1r_sb = load_const("b1r_sb", d_b1r, [128, C1])
            b2r_sb = load_const("b2r_sb", d_b2r, [128, C2])
            bcr_sb = load_const("bcr_sb", d_bcr, [G, 2])
            iotaC_sb = load_const("iotaC_sb", d_iotaC, [128, 128])
            iotaG_sb = load_const("iotaG_sb", d_iotaG, [128, G])
            ident_sb = load_const("ident_sb", d_ident, [128, 128])

            sq_sb = const.tile([128, TW], F32, name="sq_sb")
            nc.scalar.sqrt(sq_sb[:, :], deg_sb[:, :])
            dinv_sb = const.tile([128, TW], F32, name="dinv_sb")
            nc.vector.reciprocal(dinv_sb[:, :], sq_sb[:, :])
            dinv2_sb = const.tile([128, TW], F32, name="dinv2_sb")
            nc.vector.tensor_mul(dinv2_sb[:, :], dinv_sb[:, :], dinv_sb[:, :])
            W2bf_sb = const.tile([C1, C2], BF16, name="W2bf_sb")
            nc.vector.tensor_copy(W2bf_sb[:, :], W2_sb[:, :])
            identb_sb = const.tile([128, 128], BF16, name="identb_sb")
            nc.vector.tensor_copy(identb_sb[:, :], ident_sb[:, :])

            u1_cc = dram.tile([NPCP, C1], BF16, name="u1_cc")
            u1_tab = dram.tile([NTOT, C1], BF16, name="u1_tab",
                               addr_space="Shared")
            u2_cc = dram.tile([NPCP, C1], BF16, name="u2_cc")
            u2_tab = dram.tile([NTOT, C1], BF16, name="u2_tab",
                               addr_space="Shared")
            ar_in = dram.tile([G, C2 + 1], F32, name="ar_in")
            ar_out = dram.tile([G, C2 + 1], F32, name="ar_out")

            # self-loop flush terms: f1 = dinv^2*(x@W1) + b1,
            # f2 = dinv*(u2_self @ W2) + b2  (built during the layer-1 flush)
            f1_all = const.tile([128, TW * C1], F32, name="f1_all")
            f2_all = const.tile([128, TW * C2], F32, name="f2_all")

            # ---------------- Phase A: u1 = dinv * (x @ W1)
            with tc.tile_pool(name="xp", bufs=3) as xp, \
                 tc.tile_pool(name="psA", bufs=4, space="PSUM") as psA, \
                 tc.tile_pool(name="sbA", bufs=4) as sbA:
                for t in range(TW):
                    xt = xp.tile([IN, 128], F32, name="xt", tag="xt")
                    nc.sync.dma_start(out=xt[:, :],
                                      in_=d_xT[:, t * 128:(t + 1) * 128])
                    hp = psA.tile([128, C1], F32, name="hp", tag="hp")
                    nc.tensor.matmul(out=hp[:, :], lhsT=xt[:, :],
                                     rhs=W1_sb[:, :], start=True, stop=True)
                    u1t = sbA.tile([128, C1], BF16, name="u1t", tag="u1t")
                    nc.vector.tensor_scalar_mul(u1t[:, :], hp[:, :],
                                                dinv_sb[:, t:t + 1])
                    nc.sync.dma_start(out=u1_cc[t * 128:(t + 1) * 128, :],
                                      in_=u1t[:, :])
                    nc.vector.scalar_tensor_tensor(
                        out=f1_all[:, t * C1:(t + 1) * C1], in0=hp[:, :],
                        scalar=dinv2_sb[:, t:t + 1], in1=b1r_sb[:, :],
                        op0=ALU.mult, op1=ALU.add)

            nc.gpsimd.collective_compute(
                "AllGather", ALU.bypass, replica_groups=rg,
                ins=[u1_cc.opt()], outs=[u1_tab.opt()])

            # ---------------- shared per-layer edge aggregation
            # m_as_lhsT=True:  out S  [128n, C1] = M^T @ g   (node-major)
            # m_as_lhsT=False: out S^T [C1, 128n] = g^T @ M  (channel-major)
            def layer_pass(tab, flush_cb, tag, m_as_lhsT):
                with tc.tile_pool(name=f"gp{tag}", bufs=16) as gpool, \
                     tc.tile_pool(name=f"mp{tag}", bufs=4) as mpool, \
                     tc.tile_pool(name=f"pst{tag}", bufs=2, space="PSUM") as pst, \
                     tc.tile_pool(name=f"fl{tag}_ps1", bufs=2, space="PSUM") as fps1, \
                     tc.tile_pool(name=f"fl{tag}_ps2", bufs=2, space="PSUM") as fps2, \
                     tc.tile_pool(name=f"fl{tag}_sb", bufs=4) as fsb:
                    for w in range(TW):
                        et = int(ET_w[w])
                        wtb = int(tile_base[w])
                        Msb = mpool.tile([128, ETMAX * 128], BF16,
                                         name="Msb", tag="M")
                        gts = []
                        for k in range(et):
                            gt = gpool.tile([128, C1], BF16, name="gt",
                                            tag="g")
                            nc.gpsimd.indirect_dma_start(
                                out=gt[:, :], out_offset=None, in_=tab,
                                in_offset=IndirectOffsetOnAxis(
                                    ap=srcidx_sb[:, wtb + k:wtb + k + 1],
                                    axis=0))
                            gts.append(gt)
                            nc.vector.tensor_scalar(
                                out=Msb[:, k * 128:(k + 1) * 128],
                                in0=iotaC_sb[:, :],
                                scalar1=dstl_sb[:, wtb + k:wtb + k + 1],
                                scalar2=None, op0=ALU.is_equal)
                        shape = [128, C1] if m_as_lhsT else [C1, 128]
                        ST = pst.tile(shape, F32, name="ST", tag="ST")
                        for k in range(et):
                            if m_as_lhsT:
                                nc.tensor.matmul(
                                    out=ST[:, :],
                                    lhsT=Msb[:, k * 128:(k + 1) * 128],
                                    rhs=gts[k][:, :],
                                    start=(k == 0), stop=(k == et - 1))
                            else:
                                nc.tensor.matmul(
                                    out=ST[:, :], lhsT=gts[k][:, :],
                                    rhs=Msb[:, k * 128:(k + 1) * 128],
                                    start=(k == 0), stop=(k == et - 1))
                        flush_cb(w, ST, fps1, fps2, fsb)

            # ---------------- layer 1 flush -> u2 (+ f2 for layer 2)
            def flush1(w, S, fps1, fps2, fsb):
                # S is node-major [128, C1]
                h1p = fsb.tile([128, C1], F32, name="h1p", tag="h1p")
                nc.vector.scalar_tensor_tensor(
                    out=h1p[:, :], in0=S[:, :], scalar=dinv_sb[:, w:w + 1],
                    in1=f1_all[:, w * C1:(w + 1) * C1],
                    op0=ALU.mult, op1=ALU.add)
                u2t = fsb.tile([128, C1], BF16, name="u2t", tag="u2t")
                nc.scalar.activation(u2t[:, :], h1p[:, :], AF.Relu,
                                     scale=dinv_sb[:, w:w + 1])
                nc.sync.dma_start(out=u2_cc[w * 128:(w + 1) * 128, :],
                                  in_=u2t[:, :])
                u2T = fps2.tile([C1, 128], BF16, name="u2T", tag="u2T")
                nc.tensor.transpose(u2T[:, :], u2t[:, :],
                                    identb_sb[:128, :128])
                u2Ts = fsb.tile([C1, 128], BF16, name="u2Ts", tag="u2Ts")
                nc.scalar.copy(u2Ts[:, :], u2T[:, :])
                f2p = fps1.tile([128, C2], F32, name="f2p", tag="f2p")
                nc.tensor.matmul(out=f2p[:, :], lhsT=u2Ts[:, :],
                                 rhs=W2bf_sb[:, :], start=True, stop=True)
                nc.vector.scalar_tensor_tensor(
                    out=f2_all[:, w * C2:(w + 1) * C2], in0=f2p[:, :],
                    scalar=dinv_sb[:, w:w + 1], in1=b2r_sb[:, :],
                    op0=ALU.mult, op1=ALU.add)

            layer_pass(u1_tab, flush1, "1", m_as_lhsT=True)

            nc.gpsimd.collective_compute(
                "AllGather", ALU.bypass, replica_groups=rg,
                ins=[u2_cc.opt()], outs=[u2_tab.opt()])

            # ---------------- layer 2 flush -> h2 -> pooling
            with tc.tile_pool(name="sbacc", bufs=1) as sbacc, \
                 tc.tile_pool(name="pspw", bufs=2, space="PSUM") as pspw:
                pacc_sb = sbacc.tile([G, C2 + 1], F32, name="pacc_sb")
                nc.vector.memset(pacc_sb[:, :], 0.0)

                def flush2(w, ST, fps1, fps2, fsb):
                    # ST is channel-major [C1, 128]
                    stsb2 = fsb.tile([C1, 128], BF16, name="stsb2",
                                     tag="stsb2")
                    nc.scalar.copy(stsb2[:, :], ST[:, :])
                    h2p = fps1.tile([128, C2], F32, name="h2p", tag="h2p")
                    nc.tensor.matmul(out=h2p[:, :], lhsT=stsb2[:, :],
                                     rhs=W2bf_sb[:, :], start=True, stop=True)
                    h2pre = fsb.tile([128, C2], F32, name="h2pre", tag="h2pre")
                    nc.vector.scalar_tensor_tensor(
                        out=h2pre[:, :], in0=h2p[:, :],
                        scalar=dinv_sb[:, w:w + 1],
                        in1=f2_all[:, w * C2:(w + 1) * C2],
                        op0=ALU.mult, op1=ALU.add)
                    h2e = fsb.tile([128, C2 + 1], BF16, name="h2e", tag="h2e")
                    nc.scalar.activation(h2e[:, :C2], h2pre[:, :], AF.Relu)
                    nc.vector.memset(h2e[:, C2:C2 + 1], 1.0)
                    Bsb = fsb.tile([128, G], BF16, name="Bsb", tag="Bsb")
                    nc.vector.tensor_scalar(
                        out=Bsb[:, :], in0=iotaG_sb[:, :],
                        scalar1=batch_sb[:, w:w + 1], scalar2=None,
                        op0=ALU.is_equal)
                    pw = pspw.tile([G, C2 + 1], F32, name="pw", tag="pw")
                    nc.tensor.matmul(out=pw[:, :], lhsT=Bsb[:, :],
                                     rhs=h2e[:, :], start=True, stop=True)
                    nc.vector.tensor_add(pacc_sb[:, :], pacc_sb[:, :],
                                         pw[:, :])

                layer_pass(u2_tab, flush2, "2", m_as_lhsT=False)

                nc.sync.dma_start(out=ar_in[:, :], in_=pacc_sb[:, :])

            nc.gpsimd.collective_compute(
                "AllReduce", ALU.add, replica_groups=rg,
                ins=[ar_in.opt()], outs=[ar_out.opt()])

            # ---------------- Phase F: pooled mean + logits
            with tc.tile_pool(name="sbF", bufs=1) as sbF, \
                 tc.tile_pool(name="psF", bufs=1, space="PSUM") as psF:
                asb = sbF.tile([G, C2 + 1], F32, name="asb")
                nc.sync.dma_start(out=asb[:, :], in_=ar_out[:, :])
                cnt = sbF.tile([G, 1], F32, name="cnt")
                nc.vector.tensor_scalar(out=cnt[:, :], in0=asb[:, C2:C2 + 1],
                                        scalar1=1.0, scalar2=None, op0=ALU.max)
                rec = sbF.tile([G, 1], F32, name="rec")
                nc.vector.reciprocal(rec[:, :], cnt[:, :])
                pooled_sb = sbF.tile([G, C2], F32, name="pooled_sb")
                nc.vector.tensor_scalar(out=pooled_sb[:, :], in0=asb[:, :C2],
                                        scalar1=rec[:, :1], scalar2=None,
                                        op0=ALU.mult)
                nc.sync.dma_start(out=d_pooled[:, :], in_=pooled_sb[:, :])

                pT = psF.tile([C2, G], F32, name="pT")
                nc.tensor.transpose(pT[:, :], pooled_sb[:, :],
                                    ident_sb[:G, :G])
                pT_sb = sbF.tile([C2, G], F32, name="pT_sb")
                nc.scalar.copy(pT_sb[:, :], pT[:, :])
                lg = psF.tile([G, 2], F32, name="lg")
                nc.tensor.matmul(out=lg[:, :], lhsT=pT_sb[:, :],
                                 rhs=Wc_sb[:, :], start=True, stop=True)
                lg_sb = sbF.tile([G, 2], F32, name="lg_sb")
                nc.vector.tensor_add(lg_sb[:, :], lg[:, :], bcr_sb[:, :])
                nc.sync.dma_start(out=d_logits[:, :], in_=lg_sb[:, :])

    nc.compile()
    return nc


# ----------------------------------------------------------------------------
# Entry point
# ----------------------------------------------------------------------------

_PROGRAM_CACHE = {}


def _get_program(P):
    key = (P["N"], P["E"], P["G"], P["IN"], P["C1"], P["C2"], P["CORES"],
           P["ET_key"])
    if key not in _PROGRAM_CACHE:
        _PROGRAM_CACHE[key] = build_program(P)
    return _PROGRAM_CACHE[key]


def run(inputs, cfg=None, trace=False):
    P = derived(cfg or full_cfg())
    in_maps, P = preprocess(inputs, P)
    nc = _get_program(P)
    res = run_bass_kernel_spmd(nc, in_maps, core_ids=list(range(P["CORES"])),
                               trace=trace)
    logits = np.asarray(res.results[0]["logits"], dtype=np.float32)
    pooled = np.asarray(res.results[0]["pooled"], dtype=np.float32)
    return (logits, pooled), res


def kernel(**inputs):
    (logits, pooled), _ = run(inputs)
    return logits, pooled
